# revision 12
# baseline (speedup 1.0000x reference)
"""Trainium2 Bass kernel for nn_ConvDecoderLayer (conv-GLU + cross-attention
decoder layer).

Strategy: pure data-parallel over batch B=8 across the 8 NeuronCores (one
batch element per core, params replicated, no collectives).

Per-core layout strategy:
  - Activations kept feature-major ("fm": [d on partitions, t free]) for all
    matmuls (PE contracts along partitions) and token-major ("tm":
    [t on partitions, d free]) for layernorm / softmax (free-dim reductions).
  - All layout flips go through the DMA xbar transpose (dma_start_transpose),
    not the PE, so TensorE cycles are spent only on real matmuls.
  - Causal K=3 conv is 3 shifted matmuls accumulating in PSUM; the shift is a
    free-dim offset into a zero-padded feature-major x buffer.
  - Softmax: scores stay in [-1.5, 1.5] for this problem family (weights are
    0.02-scale), so exp needs no max subtraction; the row sum comes for free
    from the ACT engine's accum_out; normalization is a per-partition
    tensor_scalar multiply.
  - bf16 matmul operands / fp32 PSUM accumulation everywhere; LN statistics in
    fp32.

Host-side folding (exact rewrites, no approximation):
  - ln1_w/ln1_b folded into Wq/bq (the q path is the only consumer of
    h = xhat*w1+b1 besides the residual, which uses a separate h tensor).
  - score scale 1/sqrt(D) folded into Wq/bq.
  - bk dropped entirely: a per-row constant added to scores cancels in
    softmax (exp ratio), exactly.
  - bv folded into bo (softmax rows sum to 1 after normalization):
    attn@ (enc Wv^T + bv) Wo^T + bo == attn@(enc Wv^T) Wo^T + (Wo bv + bo).
"""

import os
import sys

import numpy as np

_REPO = "/opt/trn_rl_repo"
if _REPO not in sys.path and os.path.isdir(_REPO):
    sys.path.insert(0, _REPO)

import ml_dtypes  # noqa: E402

BF16_NP = ml_dtypes.bfloat16

B, T_FULL, S_FULL, D, KW = 8, 2048, 2048, 512, 3
P = 128
DC = D // P  # 4 d-chunks of 128

_BUILD_CACHE: dict = {}


def _build_program(T: int, S: int):
    """Build the single-core Bass program (identical on all 8 cores)."""
    from contextlib import ExitStack

    import concourse.bacc as bacc
    import concourse.bass as bass
    import concourse.tile as tile
    from concourse import mybir

    f32 = mybir.dt.float32
    bf16 = mybir.dt.bfloat16
    AF = mybir.ActivationFunctionType
    OP = mybir.AluOpType

    NT = T // P          # t-chunks of 128
    NS = S // P          # s-chunks of 128
    NTG = T // 512       # t-groups of 512 (projections)
    NSG = S // 512       # s-groups of 512 (scores free dim)
    NCG = T // 256       # conv t-groups of 256
    TP = T + 16          # host-padded fm free size (16 leading zero cols)

    nc = bacc.Bacc(
        "TRN2", target_bir_lowering=False, debug=False, num_devices=8
    )

    xb = nc.declare_dram_parameter("xb", [T + 16, D], bf16, False)
    encb = nc.declare_dram_parameter("encb", [S, D], bf16, False)
    wct = nc.declare_dram_parameter("wct", [KW * DC, P, 2 * D], bf16, False)
    wqt = nc.declare_dram_parameter("wqt", [DC, P, D], bf16, False)
    wkt = nc.declare_dram_parameter("wkt", [DC, P, D], bf16, False)
    wvt = nc.declare_dram_parameter("wvt", [DC, P, D], bf16, False)
    wot = nc.declare_dram_parameter("wot", [DC, P, D], bf16, False)
    convb = nc.declare_dram_parameter("convb", [1, 2 * D], bf16, False)
    bqt = nc.declare_dram_parameter("bqt", [P, DC], f32, False)
    w1r = nc.declare_dram_parameter("w1r", [1, D], bf16, False)
    b1r = nc.declare_dram_parameter("b1r", [1, D], bf16, False)
    bconst = nc.declare_dram_parameter("bconst", [1, D], bf16, False)
    w2r = nc.declare_dram_parameter("w2r", [1, D], f32, False)
    b2r = nc.declare_dram_parameter("b2r", [1, D], f32, False)
    outb = nc.declare_dram_parameter("outb", [T, D], f32, True)
    attnb = nc.declare_dram_parameter("attnb", [T, S], bf16, True)

    with tile.TileContext(nc) as tc, ExitStack() as ctx:
        consts = ctx.enter_context(tc.tile_pool(name="consts", bufs=1))
        wpool = ctx.enter_context(tc.tile_pool(name="weights", bufs=1))

        # ---- constants / small params ----
        ones_r = consts.tile([1, 512], bf16)
        nc.vector.memset(ones_r, 1.0)
        eps_t = consts.tile([P, 1], f32)
        nc.vector.memset(eps_t, 1e-5)
        convb_sb = consts.tile([1, 2 * D], bf16)
        nc.gpsimd.dma_start(out=convb_sb, in_=convb[:, :])
        bconst_sb = consts.tile([1, D], bf16)
        nc.gpsimd.dma_start(out=bconst_sb, in_=bconst[:, :])
        bq_sb = consts.tile([P, DC], f32)
        nc.gpsimd.dma_start(out=bq_sb, in_=bqt[:, :])
        w1_bc = consts.tile([P, D], bf16)
        w1_src = w1r[:, :]
        nc.gpsimd.dma_start(
            out=w1_bc,
            in_=bass.AP(tensor=w1_src.tensor, offset=w1_src.offset,
                        ap=[[0, P], [1, D]]),
        )
        b1_bc = consts.tile([P, D], bf16)
        b1_src = b1r[:, :]
        nc.gpsimd.dma_start(
            out=b1_bc,
            in_=bass.AP(tensor=b1_src.tensor, offset=b1_src.offset,
                        ap=[[0, P], [1, D]]),
        )
        w2_bc = consts.tile([P, D], f32)
        w2_src = w2r[:, :]
        nc.gpsimd.dma_start(
            out=w2_bc,
            in_=bass.AP(tensor=w2_src.tensor, offset=w2_src.offset,
                        ap=[[0, P], [1, D]]),
        )
        b2_bc = consts.tile([P, D], f32)
        b2_src = b2r[:, :]
        nc.gpsimd.dma_start(
            out=b2_bc,
            in_=bass.AP(tensor=b2_src.tensor, offset=b2_src.offset,
                        ap=[[0, P], [1, D]]),
        )

        # ---- weights (projection weights persist; conv weights phase-local) ----
        wqt_sb = wpool.tile([P, DC, D], bf16)
        wkt_sb = wpool.tile([P, DC, D], bf16)
        wvt_sb = wpool.tile([P, DC, D], bf16)
        wot_sb = wpool.tile([P, DC, D], bf16)
        for dst, src in ((wqt_sb, wqt), (wkt_sb, wkt), (wvt_sb, wvt), (wot_sb, wot)):
            for c in range(DC):
                nc.gpsimd.dma_start(out=dst[:, c, :], in_=src[c])

        # ---- persistent activation buffers ----
        xkq = ctx.enter_context(tc.tile_pool(name="xkq", bufs=1))
        k_fm = xkq.tile([P, DC, T], bf16, tag="k_fm")
        q_fm = xkq.tile([P, DC, T], bf16, tag="q_fm")
        v_tm = xkq.tile([P, NS, D], bf16, tag="v_tm")
        h_tm = xkq.tile([P, NT, D], bf16, tag="h_tm")

        with tc.tile_pool(name="fmtmp", bufs=4) as fmtmp, \
             tc.tile_pool(name="wcpool", bufs=1) as wcpool, \
             tc.tile_pool(name="stg", bufs=3) as stg:
            wct_sb = wcpool.tile([P, KW * DC, 2 * D], bf16)
            for i in range(KW * DC):
                nc.gpsimd.dma_start(out=wct_sb[:, i, :], in_=wct[i])

            x_fm = fmtmp.tile([P, DC, TP], bf16, tag="fm")
            enc_fm = fmtmp.tile([P, DC, S], bf16, tag="fm")
            pre1_fm = fmtmp.tile([P, DC, T], bf16, tag="fm")

            # x, enc: DRAM -> SBUF feature-major via xbar transpose.
            # xb is host-padded with 16 leading zero rows (causal pad), so
            # every transpose writes a full, 32B-aligned, contiguous row.
            for c in range(DC):
                nc.sync.dma_start_transpose(
                    out=x_fm[:, c, :], in_=xb[:, c * P:(c + 1) * P]
                )
                nc.sync.dma_start_transpose(
                    out=enc_fm[:, c, :], in_=encb[:, c * P:(c + 1) * P]
                )

            # ========================= conv + GLU =========================
            # conv[t, o] = sum_s sum_d W[o, d, s] x[t-2+s, d] + conv_b[o]
            # a = conv[:, :D]; g = conv[:, D:]; pre1 = a*sigmoid(g) + x
            with tc.tile_pool(name="cps", bufs=2, space=bass.MemorySpace.PSUM) as cps, \
                 tc.tile_pool(name="gluw", bufs=2) as gluw:
                for g in range(NCG):
                    t0 = g * 256
                    a_ps = cps.tile([P, DC, 256], f32, tag="a")
                    g_ps = cps.tile([P, DC, 256], f32, tag="g")
                    for oc in range(2 * DC):
                        dst = a_ps[:, oc, :] if oc < DC else g_ps[:, oc - DC, :]
                        # bias via rank-1 matmul (resets the accum region)
                        nc.tensor.matmul(
                            dst, convb_sb[:, oc * P:(oc + 1) * P], ones_r[:, 0:256],
                            start=True, stop=False,
                        )
                        n_mm = KW * DC
                        i_mm = 0
                        for s in range(KW):
                            for dc in range(DC):
                                i_mm += 1
                                nc.tensor.matmul(
                                    dst,
                                    wct_sb[:, s * DC + dc, oc * P:(oc + 1) * P],
                                    x_fm[:, dc, 14 + t0 + s: 14 + t0 + s + 256],
                                    start=False, stop=(i_mm == n_mm),
                                )
                    sig = gluw.tile([P, DC, 256], bf16, tag="sig")
                    nc.scalar.activation(sig, g_ps, AF.Sigmoid)
                    glu = gluw.tile([P, DC, 256], bf16, tag="glu")
                    nc.vector.tensor_mul(glu, a_ps, sig)
                    nc.vector.tensor_add(
                        pre1_fm[:, :, t0:t0 + 256], glu,
                        x_fm[:, :, 16 + t0:16 + t0 + 256],
                    )

            # ==================== k, v projections =======================
            with tc.tile_pool(name="pj", bufs=4, space=bass.MemorySpace.PSUM) as pj:
                # k_fm[dk, s] (feature-major)
                for kc in range(DC):
                    for sg in range(NSG):
                        ps = pj.tile([P, 512], f32, tag="pj")
                        for dc in range(DC):
                            nc.tensor.matmul(
                                ps, wkt_sb[:, dc, kc * P:(kc + 1) * P],
                                enc_fm[:, dc, sg * 512:(sg + 1) * 512],
                                start=(dc == 0), stop=(dc == DC - 1),
                            )
                        nc.vector.tensor_copy(
                            k_fm[:, kc, sg * 512:(sg + 1) * 512], ps
                        )
                # v_tm[s, dv] (token-major)
                for sc in range(NS):
                    ps = pj.tile([P, 512], f32, tag="pj")
                    for dc in range(DC):
                        nc.tensor.matmul(
                            ps, enc_fm[:, dc, sc * P:(sc + 1) * P], wvt_sb[:, dc, :],
                            start=(dc == 0), stop=(dc == DC - 1),
                        )
                    nc.scalar.copy(v_tm[:, sc, :], ps)

                # ==================== LN1 (token-major) ===================
                # pre1_fm -> token-major staging via xbar transpose, per
                # t-quarter (512 tokens = 4 t-chunks), then stats+normalize.
                xhat_fm = fmtmp.tile([P, DC, T], bf16, tag="fm")
                n_quarters = T // 512
                with tc.tile_pool(name="ln1", bufs=4) as ln1:
                    for qtr in range(n_quarters):
                        # qt[t', dc, sub, d'] = pre1[d=dc*128+d',
                        #                            t=qtr*512+sub*128+t']
                        qt = stg.tile([P, DC, 4, P], bf16, tag="qt")
                        for dc in range(DC):
                            nc.sync.dma_start_transpose(
                                out=qt[:, dc],
                                in_=pre1_fm[:, dc, qtr * 512:(qtr + 1) * 512],
                            )
                        for sub in range(4):
                            tcn = qtr * 4 + sub
                            src = qt[:, :, sub, :]
                            st = ln1.tile([P, DC, 6], f32, tag="st")
                            for dc in range(DC):
                                nc.vector.bn_stats(st[:, dc, :], qt[:, dc, sub, :])
                            mv = ln1.tile([P, 2], f32, tag="mv")
                            nc.vector.bn_aggr(mv, st)
                            rstd = ln1.tile([P, 1], f32, tag="rstd")
                            nc.scalar.activation(rstd, mv[:, 1:2], AF.Sqrt, bias=eps_t)
                            nc.vector.reciprocal(rstd, rstd)
                            # xh[t', dc*128+d'] = xhat (token-major chunk)
                            xh = ln1.tile([P, D], bf16, tag="xh")
                            nc.vector.tensor_scalar(
                                out=xh.rearrange("p (a b) -> p a b", b=P),
                                in0=src, scalar1=mv[:, 0:1], scalar2=rstd,
                                op0=OP.subtract, op1=OP.mult,
                            )
                            xst = stg.tile([P, DC, P], bf16, tag="xst")
                            nc.sync.dma_start_transpose(out=xst, in_=xh)
                            nc.vector.tensor_copy(
                                xhat_fm[:, :, tcn * P:(tcn + 1) * P], xst
                            )
                            # h = xhat*w1 + b1, token-major, on GpSimd
                            nc.gpsimd.tensor_mul(h_tm[:, tcn, :], xh, w1_bc)
                            nc.gpsimd.tensor_add(
                                h_tm[:, tcn, :], h_tm[:, tcn, :], b1_bc
                            )

                # ===================== q projection ======================
                for qc in range(DC):
                    for tg in range(NTG):
                        ps = pj.tile([P, 512], f32, tag="pj")
                        for dc in range(DC):
                            nc.tensor.matmul(
                                ps, wqt_sb[:, dc, qc * P:(qc + 1) * P],
                                xhat_fm[:, dc, tg * 512:(tg + 1) * 512],
                                start=(dc == 0), stop=(dc == DC - 1),
                            )
                        nc.vector.tensor_scalar(
                            out=q_fm[:, qc, tg * 512:(tg + 1) * 512], in0=ps,
                            scalar1=bq_sb[:, qc:qc + 1], scalar2=None, op0=OP.add,
                        )

        # ========================== attention ============================
        with tc.tile_pool(name="scps", bufs=2, space=bass.MemorySpace.PSUM) as scps, \
             tc.tile_pool(name="cxps", bufs=2, space=bass.MemorySpace.PSUM) as cxps, \
             tc.tile_pool(name="opps", bufs=2, space=bass.MemorySpace.PSUM) as opps, \
             tc.tile_pool(name="att", bufs=2) as att, \
             tc.tile_pool(name="ep", bufs=2) as ep:

            state = {}

            HALF = min(1024, S)
            NH = S // HALF

            def attn_front(tcn):
                """scores -> exp(+rowsum) -> normalize -> DMA out + transpose."""
                e_t = att.tile([P, S], bf16, tag="E")
                sums = att.tile([P, NH], f32, tag="sums")
                for half in range(NH):
                    ps = scps.tile([P, HALF], f32, tag="sc")
                    for sg in range(HALF // 512):
                        sga = half * (HALF // 512) + sg
                        for dc in range(DC):
                            nc.tensor.matmul(
                                ps[:, sg * 512:(sg + 1) * 512],
                                q_fm[:, dc, tcn * P:(tcn + 1) * P],
                                k_fm[:, dc, sga * 512:(sga + 1) * 512],
                                start=(dc == 0), stop=(dc == DC - 1),
                            )
                    nc.scalar.activation(
                        e_t[:, half * HALF:(half + 1) * HALF], ps, AF.Exp,
                        accum_out=sums[:, half:half + 1],
                    )
                ssum = att.tile([P, 1], f32, tag="ssum")
                if NH == 2:
                    nc.vector.tensor_add(ssum, sums[:, 0:1], sums[:, 1:2])
                else:
                    nc.vector.tensor_copy(ssum, sums[:, 0:1])
                rinv = att.tile([P, 1], f32, tag="rinv")
                nc.vector.reciprocal(rinv, ssum)
                # normalize in place (bf16 4x mode), then store + transpose
                nc.vector.tensor_scalar_mul(e_t, in0=e_t, scalar1=rinv)
                nc.gpsimd.dma_start(out=attnb[tcn * P:(tcn + 1) * P, :], in_=e_t)
                et_t = att.tile([P, NS, P], bf16, tag="ET")
                nc.sync.dma_start_transpose(out=et_t, in_=e_t)
                state[tcn] = et_t

            def attn_back(tcn):
                """ctx -> out-projection -> residual + LN2 -> DMA out."""
                et_t = state.pop(tcn)
                ctx_ps = cxps.tile([P, D], f32, tag="ctx")
                for sc in range(NS):
                    nc.tensor.matmul(
                        ctx_ps, et_t[:, sc, :], v_tm[:, sc, :],
                        start=(sc == 0), stop=(sc == NS - 1),
                    )
                ctx_sb = ep.tile([P, D], bf16, tag="ctx_sb")
                nc.vector.tensor_copy(ctx_sb, ctx_ps)
                ctx_fm = ep.tile([P, DC, P], bf16, tag="ctx_fm")
                nc.sync.dma_start_transpose(out=ctx_fm, in_=ctx_sb)
                op_ps = opps.tile([P, D], f32, tag="op")
                for dc in range(DC):
                    nc.tensor.matmul(
                        op_ps, ctx_fm[:, dc, :], wot_sb[:, dc, :],
                        start=(dc == 0), stop=False,
                    )
                nc.tensor.matmul(
                    op_ps, ones_r[:, 0:P], bconst_sb[:, :], start=False, stop=True,
                )
                pre2 = ep.tile([P, D], f32, tag="pre2")
                nc.vector.tensor_add(pre2, h_tm[:, tcn, :], op_ps)
                st = ep.tile([P, 6], f32, tag="st2")
                nc.vector.bn_stats(st, pre2)
                mv = ep.tile([P, 2], f32, tag="mv2")
                nc.vector.bn_aggr(mv, st)
                rstd = ep.tile([P, 1], f32, tag="rstd2")
                nc.scalar.activation(rstd, mv[:, 1:2], AF.Sqrt, bias=eps_t)
                nc.vector.reciprocal(rstd, rstd)
                nc.vector.tensor_scalar(
                    out=pre2, in0=pre2, scalar1=mv[:, 0:1], scalar2=rstd,
                    op0=OP.subtract, op1=OP.mult,
                )
                nc.gpsimd.tensor_mul(pre2, pre2, w2_bc)
                nc.gpsimd.tensor_add(pre2, pre2, b2_bc)
                nc.gpsimd.dma_start(out=outb[tcn * P:(tcn + 1) * P, :], in_=pre2)

            for tcn in range(NT + 1):
                if tcn < NT:
                    attn_front(tcn)
                if tcn >= 1:
                    attn_back(tcn - 1)

    nc.compile()
    return nc


def _prep_weights(inputs):
    """Host-side folding + bf16 casting of the replicated parameters."""
    f32 = np.float32
    conv_w = np.asarray(inputs["conv_w"], f32)
    conv_b = np.asarray(inputs["conv_b"], f32)
    ln1_w = np.asarray(inputs["ln1_w"], f32)
    ln1_b = np.asarray(inputs["ln1_b"], f32)
    Wq = np.asarray(inputs["Wq"], f32)
    bq = np.asarray(inputs["bq"], f32)
    Wk = np.asarray(inputs["Wk"], f32)
    Wv = np.asarray(inputs["Wv"], f32)
    bv = np.asarray(inputs["bv"], f32)
    Wo = np.asarray(inputs["Wo"], f32)
    bo = np.asarray(inputs["bo"], f32)
    ln2_w = np.asarray(inputs["ln2_w"], f32)
    ln2_b = np.asarray(inputs["ln2_b"], f32)

    sc = 1.0 / np.sqrt(np.float32(D))
    Wq_p = Wq * ln1_w[None, :] * sc            # [o, d]
    bq_p = (Wq @ ln1_b + bq) * sc              # [o]
    bconst = bo + Wo @ bv                      # [o]

    def chunks_T(W):  # W [o, d] -> W.T chunked [DC, P, D] (d on partitions)
        return np.ascontiguousarray(
            W.T.reshape(DC, P, W.shape[0]).astype(BF16_NP)
        )

    wct = np.empty((KW * DC, P, 2 * D), dtype=BF16_NP)
    for s in range(KW):
        WsT = conv_w[:, :, s].T  # [d, o]
        for dc in range(DC):
            wct[s * DC + dc] = WsT[dc * P:(dc + 1) * P, :].astype(BF16_NP)

    return {
        "wct": wct,
        "wqt": chunks_T(Wq_p),
        "wkt": chunks_T(Wk),
        "wvt": chunks_T(Wv),
        "wot": chunks_T(Wo),
        "convb": conv_b.reshape(1, 2 * D).astype(BF16_NP),
        "bqt": np.ascontiguousarray(bq_p.reshape(DC, P).T).astype(f32),
        "w1r": ln1_w.reshape(1, D).astype(BF16_NP),
        "b1r": ln1_b.reshape(1, D).astype(BF16_NP),
        "bconst": bconst.reshape(1, D).astype(BF16_NP),
        "w2r": ln2_w.reshape(1, D).astype(f32),
        "b2r": ln2_b.reshape(1, D).astype(f32),
    }


LAST_RESULTS = None  # BassKernelResults of the most recent run (for profiling)


def kernel(**inputs):
    global LAST_RESULTS
    from concourse.bass_utils import run_bass_kernel_spmd

    x = np.asarray(inputs["x"], np.float32)
    enc = np.asarray(inputs["enc"], np.float32)
    b = x.shape[0]
    T, S = x.shape[1], enc.shape[1]

    key = (T, S)
    if key not in _BUILD_CACHE:
        _BUILD_CACHE[key] = _build_program(T, S)
    nc = _BUILD_CACHE[key]

    wmap = _prep_weights(inputs)
    in_maps = []
    for i in range(b):
        m = dict(wmap)
        xpad = np.zeros((T + 16, x.shape[2]), np.float32)
        xpad[16:] = x[i]
        m["xb"] = xpad.astype(BF16_NP)
        m["encb"] = enc[i].astype(BF16_NP)
        in_maps.append(m)

    trace = bool(os.environ.get("BASS_KERNEL_TRACE"))
    res = run_bass_kernel_spmd(nc, in_maps, list(range(b)), trace=trace)
    LAST_RESULTS = res
    out = np.stack([np.asarray(r["outb"], np.float32) for r in res.results])
    attn = np.stack([np.asarray(r["attnb"]).astype(np.float32) for r in res.results])
    return out, attn
